# revision 47
# baseline (speedup 1.0000x reference)
"""GAT (2-layer, DGL-style) Bass kernel for 8 Trainium2 NeuronCores — v2.

Contract: kernel(**inputs) takes FULL unsharded inputs and returns the FULL
[N, NCLS] float32 output.

v2 design (vs v1 indirect-DMA baseline):
- Edge gathers use batched `dma_gather` (one SWDGE instruction per
  (window-batch, idx-chunk)) instead of one indirect DMA per 128 edges:
  Pool-engine descriptor-generation drops ~10x.
- Tables are bf16 with 256B rows (dma_gather requires row size % 256B == 0).
  Layer-1 rows hold B@h where B is block-diagonal per head with first row
  al1[h]: el_h = (Bh)[16h] rides inside the 128 cols for free.  Per window the
  aggregated messages are un-transformed with one 128x128 matmul by B^-1
  (block-orthogonal completion keeps it well conditioned).
- er values (needed only for the core's own dst windows) are computed into
  SBUF-resident buffers; never gathered or re-read from DRAM.
- All edge-phase matmuls (one-hot scatter, er expansion, broadcasts) run in
  bf16: 1 PE cycle/row instead of 4 for fp32.
- exp(leaky_relu(e)) uses ACT Lrelu+Exp.
"""

import math
from contextlib import ExitStack
from dataclasses import dataclass, field

import numpy as np
import ml_dtypes

from concourse import bacc, bass, mybir, tile
from concourse.bass_utils import run_bass_kernel_spmd
from concourse import library_config

f32 = mybir.dt.float32
bf16 = mybir.dt.bfloat16
i16 = mybir.dt.int16
ALU = mybir.AluOpType
ACTF = mybir.ActivationFunctionType

P = 128
NCH = 4  # idx chunks (int16 limit)
CHROWS = 25000  # rows per chunk
TB = 96  # target tiles per window batch


@dataclass
class Cfg:
    N: int = 100000
    E: int = 1600000
    IN: int = 256
    HID: int = 128
    HEADS: int = 8
    DH: int = 16
    NCLS: int = 64
    C: int = 8
    SLOPE: float = 0.2

    @property
    def S(self):
        return self.N // self.C

    @property
    def NW(self):
        return (self.S + P - 1) // P


@dataclass
class Sched:
    t: list  # [NW][NCH] tile counts (common across cores)
    k: list  # [NW] total tiles per window
    batches: list  # list of window lists
    batch_start: list  # [nb] global tile offset of batch
    chunk_ofs: list  # [nb][NCH] global tile offset of chunk block
    chunk_nt: list  # [nb][NCH] tiles in chunk block
    tile_pos: list  # [NW][NCH] global tile index of window-chunk run
    loc: list  # [NW][NCH] local tile offset within window
    Ttot: int
    DTW: int  # max k*P (dstlT row width)
    wstart: list = None  # [NW] window-local global tile offsets


def make_B(al1, HEADS, DH):
    HID = HEADS * DH
    B = np.zeros((HID, HID))
    Binv = np.zeros((HID, HID))
    for h in range(HEADS):
        a = al1[h].astype(np.float64)
        na = np.linalg.norm(a)
        M = np.eye(DH)
        M[:, 0] = a / na
        Q, _ = np.linalg.qr(M)
        if np.dot(Q[:, 0], a) < 0:
            Q[:, 0] = -Q[:, 0]
        blk = Q.T.copy()
        blk[0] *= na
        blkinv = np.linalg.inv(blk)
        B[h * DH : (h + 1) * DH, h * DH : (h + 1) * DH] = blk
        Binv[h * DH : (h + 1) * DH, h * DH : (h + 1) * DH] = blkinv
    return B, Binv


def host_prep(cfg: Cfg, src: np.ndarray, dst: np.ndarray):
    """Sort/partition edges by (dst shard, dst window, src chunk); produce a
    tile schedule common to all cores plus per-core packed idx/dstl arrays."""
    S, NW, C = cfg.S, cfg.NW, cfg.C
    src = np.asarray(src).astype(np.int64)
    dst = np.asarray(dst).astype(np.int64)
    shard = dst // S

    per_core = []
    cnt = np.zeros((C, NW, NCH), np.int64)
    for c in range(C):
        m = shard == c
        s_c = src[m]
        d_c = dst[m] - c * S
        w = d_c >> 7
        ch = s_c // CHROWS
        order = np.lexsort((ch, w))
        s_c, d_c, w, ch = s_c[order], d_c[order], w[order], ch[order]
        np.add.at(cnt[c], (w, ch), 1)
        per_core.append((s_c, d_c))

    t = np.ceil(cnt.max(axis=0) / P).astype(np.int64)  # [NW, NCH]
    k = t.sum(axis=1)  # [NW]

    # batches
    batches = []
    cur, cur_t = [], 0
    for w in range(NW):
        if cur and cur_t + k[w] > TB:
            batches.append(cur)
            cur, cur_t = [], 0
        cur.append(w)
        cur_t += int(k[w])
    if cur:
        batches.append(cur)

    nb = len(batches)
    batch_start = [0] * nb
    chunk_ofs = [[0] * NCH for _ in range(nb)]
    chunk_nt = [[0] * NCH for _ in range(nb)]
    tile_pos = [[0] * NCH for _ in range(NW)]
    loc = [[0] * NCH for _ in range(NW)]
    g = 0
    for b, wins in enumerate(batches):
        batch_start[b] = g
        for ch in range(NCH):
            chunk_ofs[b][ch] = g
            for w in wins:
                tile_pos[w][ch] = g
                g += int(t[w][ch])
            chunk_nt[b][ch] = g - chunk_ofs[b][ch]
    Ttot = g
    for w in range(NW):
        o = 0
        for ch in range(NCH):
            loc[w][ch] = o
            o += int(t[w][ch])
    DTW = int(k.max()) * P

    wstart = [0] * NW
    acc = 0
    for w in range(NW):
        wstart[w] = acc
        acc += int(k[w])

    sched = Sched(
        t=t.tolist(), k=[int(x) for x in k], batches=batches,
        batch_start=batch_start, chunk_ofs=chunk_ofs, chunk_nt=chunk_nt,
        tile_pos=tile_pos, loc=loc, Ttot=Ttot, DTW=DTW, wstart=wstart,
    )

    # per-core packs
    packs = []
    for c in range(C):
        s_c, d_c = per_core[c]
        idx_flat = np.zeros(Ttot * P, np.int16)
        dstl_flat = np.full(Ttot * P, -1.0, np.float32)
        # walk (w, ch) groups in sorted edge order
        pos = 0
        for w in range(NW):
            for ch in range(NCH):
                n = int(cnt[c, w, ch])
                gt = tile_pos[w][ch]
                nt = int(t[w][ch])
                if nt == 0:
                    assert n == 0
                    continue
                sl = slice(gt * P, gt * P + n)
                idx_flat[sl] = (s_c[pos : pos + n] - ch * CHROWS).astype(np.int16)
                dstl_flat[sl] = (d_c[pos : pos + n] - w * P).astype(np.float32)
                pos += n
        assert pos == len(s_c)

        # idx_pack [128, Ttot*8]: global slot i -> [i%16 + 16*rep, i//16]
        ii = np.arange(Ttot * P)
        idx_pack = np.zeros((P, Ttot * 8), np.int16)
        idx_pack[ii % 16, ii // 16] = idx_flat
        for rep in range(1, 8):
            idx_pack[rep * 16 : rep * 16 + 16] = idx_pack[0:16]
        # window-local flat dstl (runs concatenated in chunk order per window)
        dstlw_flat = np.full(Ttot * P, -1.0, np.float32)
        for w in range(NW):
            wl = wstart[w] * P
            for ch in range(NCH):
                gt, nt = tile_pos[w][ch], int(t[w][ch])
                if nt:
                    dstlw_flat[wl : wl + nt * P] = dstl_flat[gt * P : (gt + nt) * P]
                    wl += nt * P
        # dstlw_pack [128, Ttot] bf16: window-local slot 128t+p -> [p, t]
        dstlw_pack = np.ascontiguousarray(
            dstlw_flat.reshape(Ttot, P).T
        ).astype(ml_dtypes.bfloat16)
        # host-built transposed one-hots: OHT[s, i] = (dstlw[i] == s), bf16
        oht_pack = (
            dstlw_flat[None, :] == np.arange(P, dtype=np.float32)[:, None]
        ).astype(ml_dtypes.bfloat16)
        packs.append((idx_pack, dstlw_pack, oht_pack))
    return sched, packs


def _ap(base: bass.AP, extra_offset_elems: int, dims):
    return bass.AP(
        tensor=base.tensor,
        offset=base.offset + extra_offset_elems,
        ap=[list(base.ap[0])] + [list(d) for d in dims],
    )


def build_program(cfg: Cfg, sched: Sched, repeat: int = 1, debug_stage: int = 0):
    nc = bacc.Bacc(
        "TRN2",
        target_bir_lowering=False,
        debug=False,
        enable_asserts=False,
        num_devices=cfg.C,
    )
    S, NW, HID, HEADS, DH, NCLS = cfg.S, cfg.NW, cfg.HID, cfg.HEADS, cfg.DH, cfg.NCLS
    Ttot, DTW = sched.Ttot, sched.DTW
    F2 = NCLS + 2  # w2ext cols: h2 | el2 | er2

    # ---- I/O ----
    xTb_d = nc.dram_tensor("xTb", [cfg.IN, cfg.N], bf16, kind="ExternalInput").ap()
    w1b_d = nc.dram_tensor("w1b", [cfg.IN, HID], bf16, kind="ExternalInput").ap()
    w1ar_d = nc.dram_tensor("w1ar", [cfg.IN, HEADS], bf16, kind="ExternalInput").ap()
    binvT_d = nc.dram_tensor("binvT", [HID, HID], f32, kind="ExternalInput").ap()
    ident32_d = nc.dram_tensor("ident32", [P, P], f32, kind="ExternalInput").ap()
    w2ext_d = nc.dram_tensor("w2ext", [HID, F2], bf16, kind="ExternalInput").ap()
    b1col_d = nc.dram_tensor("b1col", [HID, 1], f32, kind="ExternalInput").ap()
    b2mat_d = nc.dram_tensor("b2mat", [P, NCLS], f32, kind="ExternalInput").ap()
    iotaf_d = nc.dram_tensor("iotaf", [P, P], bf16, kind="ExternalInput").ap()
    ident_d = nc.dram_tensor("ident", [P, P], bf16, kind="ExternalInput").ap()
    idx_d = nc.dram_tensor("idx_pack", [P, Ttot * 8], i16, kind="ExternalInput").ap()
    dstlw_d = nc.dram_tensor("dstlw_pack", [P, Ttot], bf16, kind="ExternalInput").ap()
    ohtT_d = nc.dram_tensor("oht_pack", [P, Ttot * P], bf16, kind="ExternalInput").ap()
    out_d = nc.dram_tensor("out", [S, NCLS], f32, kind="ExternalOutput").ap()

    # ---- internal DRAM ----
    table1_d = nc.dram_tensor("table1", [cfg.N, HID], bf16).ap()
    h2sh_d = nc.dram_tensor("h2sh", [S, P], bf16).ap()
    h2full_d = nc.dram_tensor("h2full", [cfg.N, P], bf16, addr_space="Shared").ap()
    if debug_stage:
        dbg1_d = nc.dram_tensor("dbg_tbl1", [2048, HID], bf16,
                                kind="ExternalOutput").ap()
        dbg2_d = nc.dram_tensor("dbg_h2sh", [S, P], bf16,
                                kind="ExternalOutput").ap()
        dbg3_d = nc.dram_tensor("dbg_er", [P, NW * 2 * HEADS + NW * 2], f32,
                                kind="ExternalOutput").ap()

    with tile.TileContext(nc) as tc, ExitStack() as octx:
        nc.gpsimd.load_library(library_config.mlp)
        const = octx.enter_context(tc.tile_pool(name="const", bufs=1))

        # ---- constants (DMA'd once) ----
        iota_f = const.tile([P, P], bf16)
        nc.sync.dma_start(out=iota_f[:], in_=iotaf_d[:, :])
        identity = const.tile([P, P], bf16)
        nc.sync.dma_start(out=identity[:], in_=ident_d[:, :])
        ident32 = const.tile([P, P], f32)
        nc.sync.dma_start(out=ident32[:], in_=ident32_d[:, :])
        binvT = const.tile([HID, HID], f32)
        nc.sync.dma_start(out=binvT[:], in_=binvT_d[:, :])
        w2ext = const.tile([HID, F2], bf16)
        nc.sync.dma_start(out=w2ext[:], in_=w2ext_d[:, :])
        b1col = const.tile([HID, 1], f32)
        nc.sync.dma_start(out=b1col[:], in_=b1col_d[:, :])
        b2mat = const.tile([P, NCLS], f32)
        nc.sync.dma_start(out=b2mat[:], in_=b2mat_d[:, :])
        idx_sb = const.tile([P, Ttot * 8], i16)
        nc.sync.dma_start(out=idx_sb[:], in_=idx_d[:, :])
        dstlw_sb = const.tile([P, Ttot], bf16)
        nc.sync.dma_start(out=dstlw_sb[:], in_=dstlw_d[:, :])
        w1b_sb = const.tile([P, 2 * HID], bf16)  # two IN-chunks side by side
        for kc in range(2):
            nc.sync.dma_start(
                out=w1b_sb[:, kc * HID : (kc + 1) * HID],
                in_=w1b_d[kc * P : (kc + 1) * P, :],
            )
        w1ar_sb = const.tile([P, 2 * HEADS], bf16)
        for kc in range(2):
            nc.sync.dma_start(
                out=w1ar_sb[:, kc * HEADS : (kc + 1) * HEADS],
                in_=w1ar_d[kc * P : (kc + 1) * P, :],
            )
        # er values as bf16 hi+lo pairs: [P, NW, 2, NH] -> cols w*2NH | +NH
        er1_keep = const.tile([P, NW * 2 * HEADS], bf16)
        nc.vector.memset(er1_keep[:], 0.0)
        er2_keep = const.tile([P, NW * 2], bf16)
        nc.vector.memset(er2_keep[:], 0.0)

        sbase = nc.partition_id() * S

        def body():
            # ---- stage A: table1[v] = B @ h(v) for all N nodes (replicated) ----
            with nc.named_scope("stage_a"), ExitStack() as actx:
                ax = actx.enter_context(tc.tile_pool(name="a_x", bufs=3))
                ast = actx.enter_context(tc.tile_pool(name="a_st", bufs=2))
                aps = actx.enter_context(tc.tile_pool(name="a_ps", bufs=3, space="PSUM"))
                GA = 512
                for g0 in range(0, cfg.N, GA):
                    gw = min(GA, cfg.N - g0)
                    xa = ax.tile([P, 2 * GA], bf16, tag="xa")
                    for kc in range(2):
                        nc.sync.dma_start(
                            out=xa[:, kc * GA : kc * GA + gw],
                            in_=xTb_d[kc * P : (kc + 1) * P, g0 : g0 + gw],
                        )
                    st = ast.tile([P, 4 * HID], bf16, tag="st")
                    ntl = (gw + P - 1) // P
                    for tl in range(ntl):
                        tw = min(P, gw - tl * P)
                        ps = aps.tile([P, HID], f32, tag="aps")
                        for kc in range(2):
                            nc.tensor.matmul(
                                ps[:tw, :],
                                lhsT=xa[:, kc * GA + tl * P : kc * GA + tl * P + tw],
                                rhs=w1b_sb[:, kc * HID : (kc + 1) * HID],
                                start=(kc == 0),
                                stop=(kc == 1),
                            )
                        nc.scalar.activation(
                            st[:tw, tl * HID : (tl + 1) * HID], ps[:tw, :], ACTF.Copy
                        )
                    if gw == GA:
                        nc.sync.dma_start(
                            out=_ap(
                                bass.AP(tensor=table1_d.tensor, offset=g0 * HID,
                                        ap=[[HID, P]]),
                                0,
                                [[P * HID, 4], [1, HID]],
                            ),
                            in_=_ap(st[:], 0, [[HID, 4], [1, HID]]),
                        )
                    else:
                        for tl in range(ntl):
                            tw = min(P, gw - tl * P)
                            nc.sync.dma_start(
                                out=table1_d[g0 + tl * P : g0 + tl * P + tw, :],
                                in_=st[:tw, tl * HID : (tl + 1) * HID],
                            )

            # ---- stage ER: er1 for own shard windows -> SBUF ----
            with nc.named_scope("stage_er"), ExitStack() as ectx:
                ex = ectx.enter_context(tc.tile_pool(name="er_x", bufs=3))
                eps = ectx.enter_context(tc.tile_pool(name="er_ps", bufs=2, space="PSUM"))
                for w in range(NW):
                    base = w * P
                    ns = min(P, S - base)
                    xe = ex.tile([P, 2 * P], bf16, tag="xe")
                    for kc in range(2):
                        nc.sync.dma_start(
                            out=xe[:, kc * P : kc * P + ns],
                            in_=xTb_d[kc * P : (kc + 1) * P, bass.ds(sbase + base, ns)],
                        )
                    pe = eps.tile([P, HEADS], f32, tag="pe")
                    for kc in range(2):
                        nc.tensor.matmul(
                            pe[:ns, :],
                            lhsT=xe[:, kc * P : kc * P + ns],
                            rhs=w1ar_sb[:, kc * HEADS : (kc + 1) * HEADS],
                            start=(kc == 0),
                            stop=(kc == 1),
                        )
                    hi = er1_keep[:ns, w * 2 * HEADS : w * 2 * HEADS + HEADS]
                    lo = er1_keep[:ns, w * 2 * HEADS + HEADS : (w + 1) * 2 * HEADS]
                    nc.vector.tensor_copy(hi, pe[:ns, :])
                    nc.vector.tensor_tensor(
                        out=lo, in0=pe[:ns, :], in1=hi, op=ALU.subtract
                    )

            # ---- edge phases ----
            def edge_phase(layer: int, ectx: ExitStack):
                if layer == 1:
                    TBL, MW, NH = table1_d, HID + HEADS, HEADS
                else:
                    TBL, MW, NH = h2full_d, NCLS + 1, 1
                WB = max(len(ws) for ws in sched.batches) + 1
                gp = ectx.enter_context(tc.tile_pool(name=f"e{layer}_g", bufs=2))
                ohp = ectx.enter_context(tc.tile_pool(name=f"e{layer}_oh", bufs=WB))
                ohtp = ectx.enter_context(tc.tile_pool(name=f"e{layer}_oht", bufs=3))
                mp = ectx.enter_context(tc.tile_pool(name=f"e{layer}_msg", bufs=3))
                sp = ectx.enter_context(tc.tile_pool(name=f"e{layer}_s", bufs=WB))
                wp = ectx.enter_context(tc.tile_pool(name=f"e{layer}_w", bufs=WB))
                ohtp2 = ohtp  # OHT now DMA-loaded; pool above
                ppse = ectx.enter_context(
                    tc.tile_pool(name=f"e{layer}_ppse", bufs=3, space="PSUM")
                )
                pacc = ectx.enter_context(
                    tc.tile_pool(name=f"e{layer}_pacc", bufs=2, space="PSUM")
                )
                if layer == 1:
                    ptt = ectx.enter_context(
                        tc.tile_pool(name="e1_ptt", bufs=1, space="PSUM")
                    )
                    ptail = ectx.enter_context(
                        tc.tile_pool(name="e1_ptail", bufs=1, space="PSUM")
                    )
                if layer == 1:
                    h2stp = ectx.enter_context(tc.tile_pool(name="h2st", bufs=2))
                else:
                    o4p = ectx.enter_context(tc.tile_pool(name="o4", bufs=2))

                stage4 = None
                stage4_w0 = 0

                for b, wins in enumerate(sched.batches):
                    bs = sched.batch_start[b]
                    bt = (sched.batch_start[b + 1] if b + 1 < len(sched.batches)
                          else Ttot) - bs
                    G = gp.tile([P, bt, P], bf16, tag="G")
                    GSPLIT = 8  # tiles per gather (SWDGE ring caps ~1024 idx)
                    for ch in range(NCH):
                        cnt_all = sched.chunk_nt[b][ch]
                        if cnt_all == 0:
                            continue
                        cbase = sched.chunk_ofs[b][ch]
                        for s0 in range(0, cnt_all, GSPLIT):
                            nt = min(GSPLIT, cnt_all - s0)
                            co = cbase + s0
                            nidx = nt * P
                            nc.gpsimd.dma_gather(
                                _ap(G[:], (co - bs) * P, [[P, nt], [1, P]]),
                                TBL[ch * CHROWS : (ch + 1) * CHROWS, :],
                                idx_sb[:, co * 8 : co * 8 + nidx // 16],
                                nidx,
                                nidx,
                                P,
                            )

                    # ---- stage-major emission over the batch's windows ----
                    def wruns(w):
                        return [
                            (sched.loc[w][ch], sched.tile_pos[w][ch] - bs,
                             sched.t[w][ch])
                            for ch in range(NCH)
                            if sched.t[w][ch] > 0
                        ]

                    OHd, PBd, RNd, CURd = {}, {}, {}, {}
                    ETd, EAd, EBd = {}, {}, {}

                    # S1: one-hot OH [edge, slot] per window (1 op via dstlw)
                    for w in wins:
                        k = sched.k[w]
                        OH = ohp.tile([P, k * P], bf16, tag="OH")
                        OHd[w] = OH
                        nc.vector.tensor_tensor(
                            out=_ap(OH[:], 0, [[P, k], [1, P]]),
                            in0=_ap(iota_f[:], 0, [[0, k], [1, P]]),
                            in1=_ap(dstlw_sb[:], sched.wstart[w], [[1, k], [0, P]]),
                            op=ALU.is_equal,
                        )

                    # S2: OHT from host (DMA); pse matmuls; et adds
                    for w in wins:
                        k = sched.k[w]
                        OHT = ohtp.tile([P, k * P], bf16, tag="OHT")
                        nc.sync.dma_start(
                            out=OHT[:],
                            in_=ohtT_d[:, sched.wstart[w] * P
                                       : (sched.wstart[w] + k) * P],
                        )
                        if layer == 1:
                            erwin = er1_keep[:, w * 2 * NH : (w + 1) * 2 * NH]
                        else:
                            erwin = er2_keep[:, w * 2 : (w + 1) * 2]
                        pse = ppse.tile([P, max(k * 2 * NH, 8)], f32, tag="pse")
                        for j in range(k):
                            nc.tensor.matmul(
                                pse[:, j * 2 * NH : (j + 1) * 2 * NH],
                                lhsT=OHT[:, j * P : (j + 1) * P],
                                rhs=erwin,
                                start=True, stop=True,
                            )
                        et = sp.tile([P, k * NH], f32, tag="et")
                        ETd[w] = et
                        if layer == 1:
                            for lo, go, nt in wruns(w):
                                nc.vector.tensor_tensor(
                                    out=_ap(et[:], lo * NH, [[NH, nt], [1, NH]]),
                                    in0=_ap(G[:], go * P, [[P, nt], [DH, NH]]),
                                    in1=_ap(pse[:], lo * 2 * NH,
                                            [[2 * NH, nt], [1, NH]]),
                                    op=ALU.add,
                                )
                                nc.vector.tensor_tensor(
                                    out=_ap(et[:], lo * NH, [[NH, nt], [1, NH]]),
                                    in0=_ap(et[:], lo * NH, [[NH, nt], [1, NH]]),
                                    in1=_ap(pse[:], lo * 2 * NH + NH,
                                            [[2 * NH, nt], [1, NH]]),
                                    op=ALU.add,
                                )
                        else:
                            for lo, go, nt in wruns(w):
                                nc.vector.tensor_tensor(
                                    out=_ap(et[:], lo, [[1, nt]]),
                                    in0=_ap(G[:], go * P + NCLS, [[P, nt]]),
                                    in1=_ap(pse[:], lo * 2, [[2, nt]]),
                                    op=ALU.add,
                                )
                                nc.vector.tensor_tensor(
                                    out=_ap(et[:], lo, [[1, nt]]),
                                    in0=_ap(et[:], lo, [[1, nt]]),
                                    in1=_ap(pse[:], lo * 2 + 1, [[2, nt]]),
                                    op=ALU.add,
                                )

                    # S3: exps (ACT)
                    for w in wins:
                        k = sched.k[w]
                        ea = sp.tile([P, k * NH], f32, tag="ea")
                        nc.scalar.activation(ea[:], ETd[w][:], ACTF.Exp)
                        eb = sp.tile([P, k * NH], f32, tag="eb")
                        nc.scalar.activation(eb[:], ETd[w][:], ACTF.Exp,
                                             scale=cfg.SLOPE)
                        EAd[w], EBd[w] = ea, eb

                    # S4: p = max(ea, eb) (DVE)
                    for w in wins:
                        k = sched.k[w]
                        pb = sp.tile([P, k * NH], bf16, tag="pb")
                        nc.vector.tensor_tensor(
                            out=pb[:], in0=EAd[w][:], in1=EBd[w][:], op=ALU.max
                        )
                        PBd[w] = pb

                    # S5: MSG (DVE) + scatter (PE) + denominators (DVE)
                    for w in wins:
                        k = sched.k[w]
                        OH, pb = OHd[w], PBd[w]
                        MSG = mp.tile([P, k * MW], bf16, tag="MSG")
                        if layer == 1:
                            for lo, go, nt in wruns(w):
                                nc.vector.tensor_tensor(
                                    out=_ap(MSG[:], lo * MW,
                                            [[MW, nt], [DH, NH], [1, DH]]),
                                    in0=_ap(G[:], go * P,
                                            [[P, nt], [DH, NH], [1, DH]]),
                                    in1=_ap(pb[:], lo * NH,
                                            [[NH, nt], [1, NH], [0, DH]]),
                                    op=ALU.mult,
                                )
                                nc.vector.tensor_copy(
                                    _ap(MSG[:], lo * MW + HID, [[MW, nt], [1, NH]]),
                                    _ap(pb[:], lo * NH, [[NH, nt], [1, NH]]),
                                )
                        else:
                            for lo, go, nt in wruns(w):
                                nc.vector.tensor_tensor(
                                    out=_ap(MSG[:], lo * MW, [[MW, nt], [1, NCLS]]),
                                    in0=_ap(G[:], go * P, [[P, nt], [1, NCLS]]),
                                    in1=_ap(pb[:], lo, [[1, nt], [0, NCLS]]),
                                    op=ALU.mult,
                                )
                                nc.vector.tensor_copy(
                                    _ap(MSG[:], lo * MW + NCLS, [[MW, nt], [1, 1]]),
                                    _ap(pb[:], lo, [[1, nt], [1, 1]]),
                                )
                        ps = pacc.tile([P, MW], f32, tag="ps")
                        for j in range(k):
                            nc.tensor.matmul(
                                ps[:, :],
                                lhsT=OH[:, j * P : (j + 1) * P],
                                rhs=MSG[:, j * MW : (j + 1) * MW],
                                start=(j == 0),
                                stop=(j == k - 1),
                            )
                        scl = sp.tile([P, NH], f32, tag="scl")
                        nc.vector.tensor_scalar(
                            out=scl[:], in0=ps[:, MW - NH : MW],
                            scalar1=1e-30, scalar2=None, op0=ALU.max,
                        )
                        rs = sp.tile([P, NH], f32, tag="rs")
                        nc.vector.reciprocal(rs[:], scl[:])
                        if layer == 1:
                            # normalized aggregate (B-domain) -> Rn, ps freed here
                            Rn = wp.tile([P, HID], f32, tag="Rn")
                            nc.vector.tensor_tensor(
                                out=_ap(Rn[:], 0, [[DH, NH], [1, DH]]),
                                in0=_ap(ps[:], 0, [[DH, NH], [1, DH]]),
                                in1=_ap(rs[:], 0, [[1, NH], [0, DH]]),
                                op=ALU.mult,
                            )
                            RNd[w] = Rn
                        else:
                            widx = w % 4
                            if widx == 0:
                                stage4 = o4p.tile([P, 4, NCLS], f32, tag="o4")
                                stage4_w0 = w
                            o2v = stage4[:, widx, :]
                            nc.vector.tensor_scalar(
                                out=o2v, in0=ps[:, 0:NCLS], scalar1=rs[:, 0:1],
                                scalar2=None, op0=ALU.mult,
                            )
                            nc.vector.tensor_tensor(
                                out=o2v, in0=o2v, in1=b2mat[:], op=ALU.add
                            )
                            if (widx == 3) or (w == NW - 1):
                                ngw = w - stage4_w0 + 1
                                if stage4_w0 * P + ngw * P <= S:
                                    nc.sync.dma_start(
                                        out=_ap(
                                            bass.AP(tensor=out_d.tensor,
                                                    offset=stage4_w0 * P * NCLS,
                                                    ap=[[NCLS, P]]),
                                            0, [[P * NCLS, ngw], [1, NCLS]],
                                        ),
                                        in_=_ap(stage4[:], 0,
                                                [[NCLS, ngw], [1, NCLS]]),
                                    )
                                else:
                                    for wi in range(ngw):
                                        wg = stage4_w0 + wi
                                        nsg = min(P, S - wg * P)
                                        nc.sync.dma_start(
                                            out=out_d[wg * P : wg * P + nsg, :],
                                            in_=stage4[:nsg, wi, :],
                                        )

                    if layer == 1:
                        # S6: un-transform + bias
                        for w in wins:
                            Rn = RNd[w]
                            pt = ptt.tile([P, P], f32, tag="pt")
                            nc.tensor.transpose(pt[:], Rn[:], ident32[:])
                            RnT = wp.tile([P, P], f32, tag="RnT")
                            nc.scalar.activation(RnT[:], pt[:], ACTF.Copy)
                            ps2 = ptail.tile([P, P], f32, tag="ps2")
                            nc.tensor.matmul(
                                ps2[:, :], lhsT=binvT[:], rhs=RnT[:],
                                start=True, stop=True,
                            )
                            cur = wp.tile([P, P], f32, tag="h1a")
                            nc.vector.tensor_scalar(
                                out=cur[:], in0=ps2[:, :], scalar1=b1col[:, 0:1],
                                scalar2=None, op0=ALU.add,
                            )
                            CURd[w] = cur
                        # S7/S8: elu twice (transposed layout, fp32)
                        for r in range(2):
                            for w in wins:
                                cur = CURd[w]
                                tmin = wp.tile([P, P], f32, tag=f"tm{r}")
                                nc.vector.tensor_scalar(
                                    out=tmin[:], in0=cur[:], scalar1=0.0,
                                    scalar2=None, op0=ALU.min,
                                )
                                nc.scalar.activation(tmin[:], tmin[:], ACTF.Exp)
                                nc.vector.tensor_scalar(
                                    out=tmin[:], in0=tmin[:], scalar1=-1.0,
                                    scalar2=None, op0=ALU.add,
                                )
                                nxt = wp.tile([P, P], f32, tag=f"he{r}")
                                nc.vector.tensor_tensor(
                                    out=nxt[:], in0=cur[:], in1=tmin[:], op=ALU.max
                                )
                                CURd[w] = nxt
                        # S9: h2 rows + er2 + staged writes
                        for w in wins:
                            base = w * P
                            ns = min(P, S - base)
                            h1T = wp.tile([P, P], bf16, tag="h1T")
                            nc.scalar.activation(h1T[:], CURd[w][:], ACTF.Copy)
                            ps3 = ptail.tile([P, F2], f32, tag="ps3")
                            nc.tensor.matmul(
                                ps3[:, :], lhsT=h1T[:], rhs=w2ext[:],
                                start=True, stop=True,
                            )
                            widx = w % 4
                            if widx == 0:
                                stage4 = h2stp.tile([P, 4, P], bf16, tag="h2st")
                                stage4_w0 = w
                                nc.vector.memset(stage4[:], 0.0)
                            nc.scalar.activation(
                                stage4[:, widx, 0 : NCLS + 1],
                                ps3[:, 0 : NCLS + 1], ACTF.Copy,
                            )
                            hi2 = er2_keep[:ns, w * 2 : w * 2 + 1]
                            lo2 = er2_keep[:ns, w * 2 + 1 : w * 2 + 2]
                            nc.vector.tensor_copy(hi2, ps3[:ns, NCLS + 1 : F2])
                            nc.vector.tensor_tensor(
                                out=lo2, in0=ps3[:ns, NCLS + 1 : F2], in1=hi2,
                                op=ALU.subtract,
                            )
                            if (widx == 3) or (w == NW - 1):
                                ngw = w - stage4_w0 + 1
                                if stage4_w0 * P + ngw * P <= S:
                                    nc.sync.dma_start(
                                        out=_ap(
                                            bass.AP(tensor=h2sh_d.tensor,
                                                    offset=stage4_w0 * P * P,
                                                    ap=[[P, P]]),
                                            0, [[P * P, ngw], [1, P]],
                                        ),
                                        in_=_ap(stage4[:], 0, [[P, ngw], [1, P]]),
                                    )
                                else:
                                    for wi in range(ngw):
                                        wg = stage4_w0 + wi
                                        nsg = min(P, S - wg * P)
                                        nc.sync.dma_start(
                                            out=h2sh_d[wg * P : wg * P + nsg, :],
                                            in_=stage4[:nsg, wi, :],
                                        )

            if debug_stage:
                with ExitStack() as dctx:
                    dbp = dctx.enter_context(tc.tile_pool(name="dbg", bufs=2))
                    for tl in range(16):
                        tmp = dbp.tile([P, HID], bf16, tag="d1")
                        nc.sync.dma_start(
                            out=tmp[:], in_=table1_d[tl * P : (tl + 1) * P, :]
                        )
                        nc.sync.dma_start(
                            out=dbg1_d[tl * P : (tl + 1) * P, :], in_=tmp[:]
                        )
            if debug_stage == 1:
                return
            with nc.named_scope("edge1"), ExitStack() as e1ctx:
                edge_phase(1, e1ctx)

            if debug_stage:
                with ExitStack() as dctx:
                    dbp = dctx.enter_context(tc.tile_pool(name="dbg2", bufs=2))
                    for wl in range(NW):
                        ns = min(P, S - wl * P)
                        tmp = dbp.tile([P, P], bf16, tag="d2")
                        nc.sync.dma_start(
                            out=tmp[:ns, :], in_=h2sh_d[wl * P : wl * P + ns, :]
                        )
                        nc.sync.dma_start(
                            out=dbg2_d[wl * P : wl * P + ns, :], in_=tmp[:ns, :]
                        )
                    er_f = dbp.tile([P, NW * 2 * HEADS + NW * 2], f32, tag="d3")
                    nc.vector.tensor_copy(er_f[:, : NW * 2 * HEADS], er1_keep[:])
                    nc.vector.tensor_copy(er_f[:, NW * 2 * HEADS :], er2_keep[:])
                    nc.sync.dma_start(out=dbg3_d[:, :], in_=er_f[:])
            if debug_stage == 2:
                return
            with nc.named_scope("allgather"):
                nc.gpsimd.collective_compute(
                    "AllGather", ALU.bypass,
                    replica_groups=[list(range(cfg.C))],
                    ins=[h2sh_d[:, :]], outs=[h2full_d[:, :]],
                )

            if debug_stage == 3:
                return
            with nc.named_scope("edge2"), ExitStack() as e2ctx:
                edge_phase(2, e2ctx)

        for _ in range(repeat):
            body()

    nc.compile()
    return nc


def build_inmaps(cfg: Cfg, x, W1, al1, ar1, b1, W2, al2, ar2, b2, packs):
    B, Binv = make_B(al1, cfg.HEADS, cfg.DH)
    W1B = (W1.astype(np.float64) @ B.T).astype(np.float32)
    W1ar = np.zeros((cfg.IN, cfg.HEADS), np.float32)
    for h in range(cfg.HEADS):
        W1ar[:, h] = W1[:, h * cfg.DH : (h + 1) * cfg.DH] @ ar1[h]
    W2ext = np.concatenate(
        [W2, W2 @ al2[0][:, None], W2 @ ar2[0][:, None]], axis=1
    ).astype(np.float32)

    bf = ml_dtypes.bfloat16
    xTb = np.ascontiguousarray(x.T).astype(bf)
    common = {
        "xTb": xTb,
        "w1b": W1B.astype(bf),
        "w1ar": W1ar.astype(bf),
        "binvT": np.ascontiguousarray(Binv.T).astype(np.float32),
        "ident32": np.eye(P, dtype=np.float32),
        "w2ext": W2ext.astype(bf),
        "b1col": b1.reshape(cfg.HID, 1).astype(np.float32),
        "b2mat": np.broadcast_to(b2, (P, cfg.NCLS)).astype(np.float32).copy(),
        "iotaf": np.broadcast_to(np.arange(P, dtype=np.float32), (P, P))
        .astype(bf).copy(),
        "ident": np.eye(P, dtype=np.float32).astype(bf),
    }
    in_maps = []
    for c in range(cfg.C):
        idx_pack, dstlw_pack, oht_pack = packs[c]
        m = dict(common)
        m["idx_pack"] = idx_pack
        m["dstlw_pack"] = dstlw_pack
        m["oht_pack"] = oht_pack
        in_maps.append(m)
    return in_maps


def run(cfg: Cfg, inputs: dict, trace: bool = False):
    x = np.asarray(inputs["x"], np.float32)
    sched, packs = host_prep(cfg, inputs["src"], inputs["dst"])
    nc = build_program(cfg, sched)
    in_maps = build_inmaps(
        cfg, x,
        np.asarray(inputs["W1"], np.float32),
        np.asarray(inputs["al1"], np.float32),
        np.asarray(inputs["ar1"], np.float32),
        np.asarray(inputs["b1"], np.float32),
        np.asarray(inputs["W2"], np.float32),
        np.asarray(inputs["al2"], np.float32),
        np.asarray(inputs["ar2"], np.float32),
        np.asarray(inputs["b2"], np.float32),
        packs,
    )
    res = run_bass_kernel_spmd(nc, in_maps, core_ids=list(range(cfg.C)), trace=trace)
    out = np.concatenate([res.results[c]["out"] for c in range(cfg.C)], axis=0)
    return out, res


def kernel(**inputs) -> np.ndarray:
    cfg = Cfg()
    out, _ = run(cfg, inputs)
    return out.astype(np.float32)
